# revision 3
# baseline (speedup 1.0000x reference)
"""Trainium2 Bass kernel v5 for nn_EnhancedQuantumLayer (6-qubit, B=32768).

Host ships the swizzled embedding-state M^T (input prep, like the existing
host-side unitary precompute); the device program per iteration is

    in-DMA (M^T, f32r) -> 8 mains (128-part, zero-padded block weights)
    -> 1 fat Square -> 8 signs -> 1 EV copy -> out-DMA

Single 8-bank PSUM tensor; ~20 instructions/iter; per-matmul cost on this
stack is ~40us fixed, so instruction count is everything.
"""
from contextlib import ExitStack

import numpy as np

import concourse.bass as bass
import concourse.mybir as mybir
from concourse.bass_utils import run_bass_kernel_spmd

F32 = mybir.dt.float32
F32R = mybir.dt.float32r
NQ = 6
NL = 6
B = 32768
NCORES = 8
BC = B // NCORES          # 4096 samples per core
SIGN_W = 512              # hw cap: matmul free dim <= 512


# ---------------------------------------------------------------- host precompute
def _host_matrices(weights):
    """(CcPacked (64,128) f32, SgnZ2 (128,6) f32) from weights (6,6,3)."""
    w = np.asarray(weights, dtype=np.float64)
    phi, theta, omega = w[..., 0], w[..., 1], w[..., 2]
    ct, st = np.cos(0.5 * theta), np.sin(0.5 * theta)
    em = np.exp(-0.5j * (phi + omega))
    ep = np.exp(0.5j * (phi + omega))
    epm = np.exp(0.5j * (phi - omega))
    emp = np.exp(-0.5j * (phi - omega))

    state = np.eye(64, dtype=np.complex128).reshape((64,) + (2,) * NQ)

    def apply_1q(state, U, q):
        ax = q + 1
        s = np.moveaxis(state, ax, -1)
        s = np.einsum('ij,...j->...i', U, s)
        return np.moveaxis(s, -1, ax)

    def cnot(state, c, t):
        ca, ta = c + 1, t + 1
        s0 = np.take(state, 0, axis=ca)
        s1 = np.take(state, 1, axis=ca)
        t_in = ta - 1 if ta > ca else ta
        s1 = np.flip(s1, axis=t_in)
        return np.stack([s0, s1], axis=ca)

    for l in range(NL):
        for q in range(NQ):
            U = np.array([
                [em[l, q] * ct[l, q], -epm[l, q] * st[l, q]],
                [emp[l, q] * st[l, q], ep[l, q] * ct[l, q]],
            ])
            state = apply_1q(state, U, q)
        r = (l % (NQ - 1)) + 1
        for q in range(NQ):
            state = cnot(state, q, (q + r) % NQ)

    stateF = state.reshape(64, 64)            # [in_e, out_o] = U[o, e]
    e = np.arange(64)
    pc = np.array([bin(v).count('1') for v in e])
    phase = (-1j) ** pc                       # (-i)^popcount: RX embedding phases
    Cc = phase[:, None] * stateF              # (64_in, 64_out)

    # device row j has qubit q at bit q; reference index e has qubit 0 as MSB
    bitrev = np.array([int(format(j, '06b')[::-1], 2) for j in range(64)])
    Cdev = Cc[bitrev, :]

    ccpacked = np.concatenate([Cdev.real, Cdev.imag], axis=1)   # (64, 128)

    o = np.arange(64)
    z = np.stack([1.0 - 2.0 * ((o >> (5 - q)) & 1) for q in range(NQ)], axis=1)
    sgn2 = np.concatenate([z, z], axis=0)                        # (128, 6)
    return ccpacked.astype(np.float32), sgn2.astype(np.float32)


def _sid(h, mc):
    """sample_local carried at mtall column mc of partition-half h."""
    sb, pl = mc >> 5, mc & 31
    s, tp, p_hi = sb >> 4, (sb >> 2) & 3, sb & 3
    return 1024 * p_hi + 32 * pl + 8 * s + 2 * tp + h


def _out_perm_v2():
    """dram out col d -> sample_local. Layout: [A-h0 | A-h1 | B-h0 | B-h1],
    chains A/B = mtall col ranges [0:1024] / [1024:2048]."""
    d = np.arange(BC)
    chain = d >> 11
    cd = d & 2047
    h = cd >> 10
    mc = chain * 1024 + (cd & 1023)
    return _sid(h, mc)


# ---------------------------------------------------------------- device program
def _build_bass(reps=1):
    nc = bass.Bass()
    mtin = nc.dram_tensor("mtin", [128, 2048], F32R, kind="ExternalInput")
    ccin = nc.dram_tensor("ccin", [128, 256], F32R, kind="ExternalInput")
    sgin = nc.dram_tensor("sgin", [128, NQ], F32R, kind="ExternalInput")
    out = nc.dram_tensor("out", [NQ, BC], F32, kind="ExternalOutput")

    ctx = ExitStack()
    with ctx:
        sb = lambda nm, shape, dt=F32R: ctx.enter_context(
            nc.sbuf_tensor(nm, shape, dt))
        ps = lambda nm, shape: ctx.enter_context(
            nc.psum_tensor(nm, shape, F32))
        sem = lambda nm: ctx.enter_context(nc.semaphore(name=nm))

        ccr = sb("ccr", [128, 256])
        sgr = sb("sgr", [128, NQ])
        mtall = [sb("mtall0", [128, 2048]), sb("mtall1", [128, 2048])]
        ppb = sb("ppb", [128, 4096])
        evb = sb("evb", [64, BC], F32)
        pAll = ps("pAll", [128, 4096])

        Sd = sem("Sd")    # weights (+32 once) then per-iter M^T DMA (+16)
        Sm = sem("Sm")    # pe mains (+8)
        Sa = sem("Sa")    # act: sq, evcp (+2)
        Sg = sem("Sg")    # pe signs (+2)
        So = sem("So")    # out dma (+16)

        sqf = mybir.ActivationFunctionType.Square
        block = ctx.enter_context(nc.Block())

        @block.sync
        def _(sync):
            sync.dma_start(out=ccr.ap()[:, :], in_=ccin[:, :]).then_inc(Sd, 16)
            sync.dma_start(out=sgr.ap()[:, :], in_=sgin[:, :]).then_inc(Sd, 16)
            for i in range(reps):
                b = i % 2
                if i >= 2:
                    # evb[b] WAR vs outDMA(i-2); reaches evcp(i) transitively
                    # via Sd -> mains -> square -> signs
                    sync.wait_ge(So, 16 * (i - 1))
                d = sync.dma_start(out=mtall[b].ap()[:, :], in_=mtin[:, :])
                if i >= 2:
                    d._wait_ge(Sm, 8 * (i - 1))   # mains(i-2) done with mtall[b]
                d.then_inc(Sd, 16)
                o_ = sync.dma_start(out=out[:, :],
                                    in_=evb.ap()[32 * b:32 * b + 6, :])
                o_._wait_ge(Sa, 2 * i + 2).then_inc(So, 16)
            sync.wait_ge(So, 16 * reps)

        @block.scalar
        def _(scalar):
            for i in range(reps):
                b = i % 2
                # ppb WAR vs signs(i-1) implied: mains(i) sit after them on PE
                q1 = nc.scalar.activation(ppb.ap()[:, :], pAll.ap()[:, :], sqf)
                q1._wait_ge(Sm, 8 * i + 8)
                q1.then_inc(Sa, 1)
                e1 = nc.scalar.copy(evb.ap()[32 * b:32 * b + 6, :],
                                    pAll.ap()[0:6, :])
                e1._wait_ge(Sg, 2 * i + 2)
                e1.then_inc(Sa, 1)

        @block.tensor
        def _(tensor):
            for i in range(reps):
                b = i % 2
                if i >= 1:
                    # pAll free only once evcp(i-1) read the EV rows
                    tensor.wait_ge(Sa, 2 * (i - 1) + 2)
                for c in range(8):
                    chain, h, half = c // 4, (c // 2) % 2, c % 2
                    mlo = 1024 * chain + 512 * half
                    plo = 2048 * chain + 1024 * h + 512 * half
                    mm = nc.tensor.matmul(
                        pAll.ap()[:, plo:plo + 512],
                        ccr.ap()[:, 128 * h:128 * h + 128],
                        mtall[b].ap()[:, mlo:mlo + 512],
                        start=True, stop=True,
                    )
                    if c == 0:
                        mm._wait_ge(Sd, 32 + 16 * (i + 1))
                    mm.then_inc(Sm, 1)
                for g in range(2):
                    nch = 2048 // SIGN_W
                    for k in range(nch):
                        lo = 2048 * g + k * SIGN_W
                        mm = nc.tensor.matmul(
                            pAll.ap()[0:6, lo:lo + SIGN_W],
                            sgr.ap()[:, :],
                            ppb.ap()[:, lo:lo + SIGN_W],
                            start=True, stop=True,
                        )
                        if g == 0 and k == 0:
                            mm._wait_ge(Sa, 2 * i + 1)   # square(i) done
                        if k == nch - 1:
                            mm.then_inc(Sg, 1)

    return nc


_CACHE = {}


def _get_nc():
    if "nc" not in _CACHE:
        _CACHE["nc"] = _build_bass()
        _CACHE["perm"] = _out_perm_v2()
    return _CACHE["nc"], _CACHE["perm"]


# ---------------------------------------------------------------- entry point
def _make_in_maps(x, weights, scale):
    x = np.asarray(x, dtype=np.float32)
    ccp, sg2 = _host_matrices(weights)
    hs = 0.5 * float(np.asarray(scale).reshape(-1)[0])
    a = x.astype(np.float64) * hs            # (B, 6) half-angles
    ca, sa = np.cos(a), np.sin(a)
    cc2 = np.zeros((128, 256), np.float32)
    cc2[0:64, 0:128] = ccp       # h0 rows x [ccp; 0]
    cc2[64:128, 128:256] = ccp   # h1 rows x [0; ccp]
    mc = np.arange(2048)
    sids = [_sid(0, mc), _sid(1, mc)]
    in_maps = []
    for k in range(NCORES):
        lo = k * BC
        # M_dev[s, j] = prod_q (bit_q(j) ? sin : cos)(a[s, q]); qubit q sits
        # at bit q of j, so stack q=5 last (outermost = MSB)
        M = np.ones((BC, 1))
        for q in range(NQ):
            M = np.stack([ca[lo:lo + BC, q, None] * M,
                          sa[lo:lo + BC, q, None] * M], axis=1).reshape(BC, -1)
        mt = np.empty((128, 2048), np.float32)
        mt[0:64] = M[sids[0]].T
        mt[64:128] = M[sids[1]].T
        in_maps.append({"mtin": mt, "ccin": cc2, "sgin": sg2})
    return in_maps


def kernel(x, weights, scale):
    nc, perm = _get_nc()
    in_maps = _make_in_maps(x, weights, scale)
    res = run_bass_kernel_spmd(nc, in_maps, list(range(NCORES))).results
    ev = np.empty((B, NQ), np.float32)
    for k in range(NCORES):
        ev[k * BC + perm, :] = res[k]["out"].T
    return ev


if __name__ == "__main__":
    import reference as ref
    import jax
    cpu = jax.devices("cpu")[0]
    with jax.default_device(cpu):
        inputs = {k: np.asarray(v) for k, v in ref.setup_inputs().items()}
        expected = np.asarray(ref.reference(**inputs))
    actual = kernel(**inputs)
    rel = np.linalg.norm(actual - expected) / np.linalg.norm(expected)
    print("max abs err:", np.abs(actual - expected).max())
    print("Relative error:", rel)
